# revision 1
# baseline (speedup 1.0000x reference)
"""Multi-head causal attention (B=4, C=2048, E=1024, H=16, D=64) on 8 trn2 cores.

Sharding: core i = (batch b=i//2, head-group g=i%2).  Each core computes its
batch's attention for 8 heads (512 features) and a partial output projection;
the host sums the two partials per batch (W_o split row-wise).

Per-core kernel (all matmuls float32r: full PE rate at N>=256, FP22 operands):
  phase 1: V = x @ Wv_g            -> [tok, 8 heads x (64 feat + ones col)]
           QT/KT per head-pair     -> [128 feat, 2048 tok]   (x.T pre-done on host)
  phase 2: per (head-pair, q-chunk 512, k-block 128):
           S^T = K^T.T @ Q^T       (row-tiled pair, K=64 contraction x 2 heads)
           W^T = exp(S^T / 8)      (one ACT over both heads' psum banks)
           diagonal causal mask    (DVE multiply with host-provided mask)
           hid/rowsum = [V|1].T @ W^T  (M=65 accumulating over k-blocks)
           normalize by 1/rowsum -> hiddenT staged to DRAM
  phase 3: out = hiddenT.T @ Wo_g  (K=512 contraction via 4 chained matmuls)
"""

import numpy as np

import concourse.bass as bass
import concourse.mybir as mybir
import concourse.tile as tile
from concourse.vector_clock import ScopedClock

B, C, E = 4, 2048, 1024
H, D = 16, 64
N_CORES = 8
GF = 512          # features per head-group (8 heads x 64)
HP = 4            # head-pairs per group
QC = 512          # q-chunk width
KB = 128          # k-block width
NQC = C // QC     # 4
NKB = C // KB     # 16
NE = E // 128     # 8 contraction tiles over E
F32 = mybir.dt.float32
F32R = mybir.dt.float32r
BF16 = mybir.dt.bfloat16

_CACHED_NC = None


class PatchedTC(tile.TileContext):
    """This walrus build caps sync waits per instruction (1 for CTRL, ~2 for
    compute ISA structs).  Hoist excess waits onto same-engine NOPs emitted
    just before the instruction (engine streams execute in order, so the
    semantics are identical), and split the end-of-kernel drain's waits
    across single-wait drain instructions."""

    WAIT_CAP = 1

    def _commit_instruction(self, inst, lazy_reg_writes=True):
        si = getattr(inst, "sync_info", None)
        if (
            si is not None
            and len(si.on_wait) > self.WAIT_CAP
            and getattr(inst, "engine", mybir.EngineType.Unassigned)
            != mybir.EngineType.Unassigned
        ):
            waits = list(si.on_wait)
            keep = waits[: self.WAIT_CAP]
            extra = waits[self.WAIT_CAP :]
            si.on_wait[:] = keep
            for w in extra:
                nop = mybir.InstNoOp(
                    name=f"I-nw{self.nc.next_id()}",
                    engine=inst.engine,
                    bass_nofuse=True,
                    sync_info=mybir.SyncInfo(on_wait=[w], on_update=[]),
                )
                super()._commit_instruction(nop, lazy_reg_writes=False)
        return super()._commit_instruction(inst, lazy_reg_writes)

    def _drain_and_barrier(self, tick_clock, wait_clock):
        carrier = self.nc.sync.drain()
        wait_clock.add_sem_waits(
            carrier.ins, ScopedClock({None: tick_clock.global_clock})
        )
        si = carrier.ins.sync_info
        waits = list(si.on_wait) if si is not None else []
        if len(waits) > 1:
            si.on_wait[:] = waits[:1]
            for w in waits[1:]:
                extra = self.nc.sync.drain()
                extra.ins.sync_info = mybir.SyncInfo(on_wait=[w], on_update=[])
        self.nc.all_engine_barrier()
        assert self.sems is not None
        popped = self.nc._tile_sem_poison_stack.pop()
        assert popped is self._sem_poison
        self.nc.clear_and_free_semaphores(list(self.sems.allocated().values()))
        self.nc.all_engine_barrier()


def build_nc():
    nc = bass.Bass("TRN2", target_bir_lowering=False)
    xT = nc.declare_dram_parameter("xT", [E, C], BF16, isOutput=False)
    Wq = nc.declare_dram_parameter("Wq", [E, GF], BF16, isOutput=False)
    Wk = nc.declare_dram_parameter("Wk", [E, GF], BF16, isOutput=False)
    Wv = nc.declare_dram_parameter("Wv", [E, GF], BF16, isOutput=False)
    Wo = nc.declare_dram_parameter("Wo", [GF, E], BF16, isOutput=False)
    msk = nc.declare_dram_parameter("mask", [128, 4 * QC], mybir.dt.bfloat16, isOutput=False)
    out = nc.declare_dram_parameter("out", [C, E], F32, isOutput=True)

    xT_t = xT.ap().rearrange("(po pi) f -> pi po f", pi=128)    # [128, 8, C]
    Wq_t = Wq.ap().rearrange("(po pi) f -> pi po f", pi=128)    # [128, 8, GF]
    Wk_t = Wk.ap().rearrange("(po pi) f -> pi po f", pi=128)
    Wv_t = Wv.ap().rearrange("(po pi) f -> pi po f", pi=128)
    Wo_t = Wo.ap().rearrange("(po pi) f -> pi po f", pi=128)    # [128, 4, E]

    with PatchedTC(nc) as tc:
        import contextlib

        with contextlib.ExitStack() as ctx:
            consts = ctx.enter_context(tc.tile_pool(name="consts", bufs=1))
            dram = ctx.enter_context(tc.tile_pool(name="dram", bufs=1, space="DRAM"))
            ppsum = ctx.enter_context(tc.tile_pool(name="ppsum", bufs=2, space="PSUM"))

            mask_sb = consts.tile([128, 4 * QC], mybir.dt.bfloat16)
            nc.sync.dma_start(mask_sb[:], msk.ap())

            xpool = ctx.enter_context(tc.tile_pool(name="xpool", bufs=1))
            vpool = ctx.enter_context(tc.tile_pool(name="vpool", bufs=1))

            xT_sb = xpool.tile([128, NE, C], BF16)
            for e in range(NE):
                nc.sync.dma_start(xT_sb[:, e, :], xT_t[:, e, :])

            # ---- phase 1a: V for all 8 heads, ones column appended per head
            with tc.tile_pool(name="wvpool", bufs=1) as wvpool:
                wv_sb = wvpool.tile([128, NE, GF], BF16)
                nc.sync.dma_start(wv_sb[:], Wv_t[:])
                v_sb = vpool.tile([128, NKB, 2 * GF], BF16)  # [tok, kb, h*(64V|64ones)]
                nc.any.memset(v_sb[:], 1.0)
                for t in range(NKB):
                    pv = ppsum.tile([128, GF], F32, tag="ppsum")
                    for e in range(NE):
                        nc.tensor.matmul(
                            pv[:],
                            lhsT=xT_sb[:, e, t * 128 : (t + 1) * 128],
                            rhs=wv_sb[:, e, :],
                            start=(e == 0),
                            stop=(e == NE - 1),
                        )
                    dst = v_sb[:, t, :].rearrange("p (h u) -> p h u", u=128)[:, :, 0:64]
                    nc.vector.tensor_copy(dst, pv[:].rearrange("p (h u) -> p h u", u=64))

            # ---- phases 1b + 2: per head-pair projections + attention
            qkpool = ctx.enter_context(tc.tile_pool(name="qkpool", bufs=2))
            wpool = ctx.enter_context(tc.tile_pool(name="wpool", bufs=1))
            stpool = ctx.enter_context(tc.tile_pool(name="stpsum", bufs=2, space="PSUM"))
            hidpool = ctx.enter_context(tc.tile_pool(name="hidpsum", bufs=1, space="PSUM"))
            wtpool = ctx.enter_context(tc.tile_pool(name="wtpool", bufs=2))
            napool = ctx.enter_context(tc.tile_pool(name="napool", bufs=2))
            hidT_dram = dram.tile([HP, 128, C], BF16)

            for hp in range(HP):
                wq_sb = wpool.tile([128, NE, 128], BF16, tag="wq")
                wk_sb = wpool.tile([128, NE, 128], BF16, tag="wk")
                nc.sync.dma_start(wq_sb[:], Wq_t[:, :, hp * 128 : (hp + 1) * 128])
                nc.sync.dma_start(wk_sb[:], Wk_t[:, :, hp * 128 : (hp + 1) * 128])
                # fp32r here: bf16 row-tiled matmul pairs crash the exec unit
                # (NRT_EXEC_UNIT_UNRECOVERABLE); fp32r pairs are stable and the
                # 2 cyc/row fp32r rate over a concurrent pair matches unpaired
                # bf16 anyway.
                qt = qkpool.tile([128, C], F32R, tag="qt")
                kt = qkpool.tile([128, C], F32R, tag="kt")
                for n in range(NQC):
                    pq = ppsum.tile([128, QC], F32, tag="ppsum")
                    for e in range(NE):
                        nc.tensor.matmul(
                            pq[:],
                            lhsT=wq_sb[:, e, :],
                            rhs=xT_sb[:, e, n * QC : (n + 1) * QC],
                            start=(e == 0),
                            stop=(e == NE - 1),
                        )
                    nc.vector.tensor_copy(qt[:, n * QC : (n + 1) * QC], pq[:])
                    pk = ppsum.tile([128, QC], F32, tag="ppsum")
                    for e in range(NE):
                        nc.tensor.matmul(
                            pk[:],
                            lhsT=wk_sb[:, e, :],
                            rhs=xT_sb[:, e, n * QC : (n + 1) * QC],
                            start=(e == 0),
                            stop=(e == NE - 1),
                        )
                    nc.vector.tensor_copy(kt[:, n * QC : (n + 1) * QC], pk[:])

                for qc in range(NQC):
                    nkb = 4 * qc + 4
                    hidA = hidpool.tile([128, QC], F32, tag="hidA")
                    hidB = hidpool.tile([128, QC], F32, tag="hidB")
                    for kb in range(nkb):
                        st = stpool.tile([128, 2 * QC], F32, tag="st")
                        nc.tensor.matmul(
                            st[:, 0:QC],
                            lhsT=kt[0:64, kb * KB : (kb + 1) * KB],
                            rhs=qt[0:64, qc * QC : (qc + 1) * QC],
                            start=True,
                            stop=True,
                        )
                        nc.tensor.matmul(
                            st[:, QC : 2 * QC],
                            lhsT=kt[64:128, kb * KB : (kb + 1) * KB],
                            rhs=qt[64:128, qc * QC : (qc + 1) * QC],
                            start=True,
                            stop=True,
                        )
                        wt = wtpool.tile([128, 2 * QC], BF16, tag="wt")
                        nc.scalar.activation(
                            wt[:], st[:], mybir.ActivationFunctionType.Exp, scale=0.125
                        )
                        dr = kb - (nkb - 4)
                        if dr >= 0:
                            nc.vector.tensor_tensor(
                                wt[:].rearrange("p (a b) -> p a b", a=2),
                                wt[:].rearrange("p (a b) -> p a b", a=2),
                                mask_sb[:, None, dr * QC : (dr + 1) * QC].to_broadcast(
                                    (128, 2, QC)
                                ),
                                mybir.AluOpType.mult,
                            )
                        # hidden rows 0:64; rowsum replicated on rows 64:128
                        # (ones columns embedded in v_sb)
                        nc.tensor.matmul(
                            hidA[:],
                            lhsT=v_sb[:, kb, 2 * hp * 128 : (2 * hp + 1) * 128],
                            rhs=wt[:, 0:QC],
                            start=(kb == 0),
                            stop=(kb == nkb - 1),
                        )
                        nc.tensor.matmul(
                            hidB[:],
                            lhsT=v_sb[:, kb, (2 * hp + 1) * 128 : (2 * hp + 2) * 128],
                            rhs=wt[:, QC : 2 * QC],
                            start=(kb == 0),
                            stop=(kb == nkb - 1),
                        )
                    # 1/rowsum via exp(-ln(rs)) on ACT: DVE's bit-exact
                    # reciprocal is ~6 cycles/elem and custom DVE ops don't
                    # compile on this toolchain; ln/exp share one table set.
                    lnA = napool.tile([64, QC], F32, tag="ln")
                    lnB = napool.tile([64, QC], F32, tag="ln")
                    recA = napool.tile([64, QC], F32, tag="rec")
                    recB = napool.tile([64, QC], F32, tag="rec")
                    nc.scalar.activation(
                        lnA[:], hidA[64:128, :], mybir.ActivationFunctionType.Ln
                    )
                    nc.scalar.activation(
                        lnB[:], hidB[64:128, :], mybir.ActivationFunctionType.Ln
                    )
                    nc.scalar.activation(
                        recA[:], lnA[:], mybir.ActivationFunctionType.Exp, scale=-1.0
                    )
                    nc.scalar.activation(
                        recB[:], lnB[:], mybir.ActivationFunctionType.Exp, scale=-1.0
                    )
                    stage = napool.tile([128, QC], BF16, tag="stage")
                    nc.vector.tensor_tensor(
                        stage[0:64, :], hidA[0:64, :], recA[:], mybir.AluOpType.mult
                    )
                    nc.vector.tensor_tensor(
                        stage[64:128, :], hidB[0:64, :], recB[:], mybir.AluOpType.mult
                    )
                    nc.sync.dma_start(
                        hidT_dram[hp, :, qc * QC : (qc + 1) * QC], stage[:]
                    )

            # ---- phase 3: out projection, contracting all 512 group features
            with tc.tile_pool(name="opool", bufs=1) as opool, tc.tile_pool(
                name="ostage", bufs=3
            ) as ostage:
                wo_sb = opool.tile([128, HP, E], BF16)
                nc.sync.dma_start(wo_sb[:], Wo_t[:])
                hf = opool.tile([128, HP, C], BF16)
                for f in range(HP):
                    nc.sync.dma_start(hf[:, f, :], hidT_dram[f, :, :])
                for qb in range(C // 128):
                    for ec in range(E // QC):
                        po = ppsum.tile([128, QC], F32, tag="ppsum")
                        for f in range(HP):
                            nc.tensor.matmul(
                                po[:],
                                lhsT=hf[:, f, qb * 128 : (qb + 1) * 128],
                                rhs=wo_sb[:, f, ec * QC : (ec + 1) * QC],
                                start=(f == 0),
                                stop=(f == HP - 1),
                            )
                        so = ostage.tile([128, QC], F32, tag="so")
                        nc.vector.tensor_copy(so[:], po[:])
                        nc.sync.dma_start(
                            out.ap()[qb * 128 : (qb + 1) * 128, ec * QC : (ec + 1) * QC],
                            so[:],
                        )
    return nc


def _make_mask():
    import ml_dtypes

    m = np.zeros((128, 4, QC), dtype=np.float32)
    for rr in range(4):
        kk = np.arange(128)[:, None]
        qq = np.arange(QC)[None, :]
        m[:, rr, :] = (128 * rr + kk <= qq).astype(np.float32)
    return np.ascontiguousarray(m.reshape(128, 4 * QC)).astype(ml_dtypes.bfloat16)


def make_in_maps(x, W_q, W_k, W_v, W_o):
    import ml_dtypes

    bf16 = ml_dtypes.bfloat16
    mask = _make_mask()
    in_maps = []
    for i in range(N_CORES):
        b, g = i // 2, i % 2
        in_maps.append(
            {
                "xT": np.ascontiguousarray(np.asarray(x)[b].T).astype(bf16),
                "Wq": np.ascontiguousarray(
                    np.asarray(W_q)[:, g * GF : (g + 1) * GF]
                ).astype(bf16),
                "Wk": np.ascontiguousarray(
                    np.asarray(W_k)[:, g * GF : (g + 1) * GF]
                ).astype(bf16),
                "Wv": np.ascontiguousarray(
                    np.asarray(W_v)[:, g * GF : (g + 1) * GF]
                ).astype(bf16),
                "Wo": np.ascontiguousarray(
                    np.asarray(W_o)[g * GF : (g + 1) * GF, :]
                ).astype(bf16),
                "mask": mask,
            }
        )
    return in_maps


def kernel(x, W_q, W_k, W_v, W_o):
    global _CACHED_NC
    from concourse.bass_utils import run_bass_kernel_spmd

    if _CACHED_NC is None:
        _CACHED_NC = build_nc()
    nc = _CACHED_NC

    in_maps = make_in_maps(x, W_q, W_k, W_v, W_o)
    res = run_bass_kernel_spmd(nc, in_maps, core_ids=list(range(N_CORES)))
    out = np.empty((B, C, E), dtype=np.float32)
    for b in range(B):
        out[b] = res.results[2 * b]["out"] + res.results[2 * b + 1]["out"]
    return out



# revision 9
# speedup vs baseline: 1.0942x; 1.0942x over previous
"""Multi-head causal attention (B=4, C=2048, E=1024, H=16, D=64) on 8 trn2 cores.

Sharding: core i = (batch b=i//2, head-group g=i%2).  Each core computes its
batch's attention for 8 heads (512 features) and a partial output projection;
the host sums the two partials per batch (W_o split row-wise).

Single software-pipelined pass, qc-outer / head-pair-inner.  The PE stream for
each attention group (qc, hp) is  [S(kb+1) | filler | AV(kb)]  so the scalar
engine's exp latency is hidden; projection matmuls (V, Q, K for upcoming
groups) and the output projection of the previous q-chunk are pumped into the
stream as filler, so the tensor engine never idles waiting on exp/normalize.
Hidden states stay in SBUF between attention and the output projection (no
DRAM roundtrip).  Diagonal k-blocks only compute the causally-valid q-window
(the S matmul window is floored at 256 columns: fp32r drops to 1/4 rate below
an output free size of 256).
"""

import contextlib
from collections import deque

import numpy as np

import concourse.bass as bass
import concourse.mybir as mybir
import concourse.tile as tile
from concourse.vector_clock import ScopedClock

B, C, E = 4, 2048, 1024
H, D = 16, 64
N_CORES = 8
GF = 512          # features per head-group (8 heads x 64)
HP = 4            # head-pairs per group
QC = 512          # q-chunk width
KB = 128          # k-block width
NQC = C // QC     # 4
NKB = C // KB     # 16
NE = E // 128     # 8 contraction tiles over E
F32 = mybir.dt.float32
F32R = mybir.dt.float32r
BF16 = mybir.dt.bfloat16

# 0: all matmuls full-width (baseline shapes, scheduling change only)
# 1: trim exp/mask/AV to the causally-valid q-window; S full width
# 2: additionally trim the fp32r S matmuls
TRIM = 0

_CACHED_NC = None


class PatchedTC(tile.TileContext):
    """This walrus build caps sync waits per instruction (1 for CTRL, ~2 for
    compute ISA structs).  Hoist excess waits onto same-engine NOPs emitted
    just before the instruction (engine streams execute in order, so the
    semantics are identical), and split the end-of-kernel drain's waits
    across single-wait drain instructions."""

    WAIT_CAP = 1

    def _commit_instruction(self, inst, lazy_reg_writes=True):
        si = getattr(inst, "sync_info", None)
        if (
            si is not None
            and len(si.on_wait) > self.WAIT_CAP
            and getattr(inst, "engine", mybir.EngineType.Unassigned)
            != mybir.EngineType.Unassigned
        ):
            waits = list(si.on_wait)
            keep = waits[: self.WAIT_CAP]
            extra = waits[self.WAIT_CAP :]
            si.on_wait[:] = keep
            for w in extra:
                nop = mybir.InstNoOp(
                    name=f"I-nw{self.nc.next_id()}",
                    engine=inst.engine,
                    bass_nofuse=True,
                    sync_info=mybir.SyncInfo(on_wait=[w], on_update=[]),
                )
                super()._commit_instruction(nop, lazy_reg_writes=False)
        return super()._commit_instruction(inst, lazy_reg_writes)

    def _drain_and_barrier(self, tick_clock, wait_clock):
        carrier = self.nc.sync.drain()
        wait_clock.add_sem_waits(
            carrier.ins, ScopedClock({None: tick_clock.global_clock})
        )
        si = carrier.ins.sync_info
        waits = list(si.on_wait) if si is not None else []
        if len(waits) > 1:
            si.on_wait[:] = waits[:1]
            for w in waits[1:]:
                extra = self.nc.sync.drain()
                extra.ins.sync_info = mybir.SyncInfo(on_wait=[w], on_update=[])
        self.nc.all_engine_barrier()
        assert self.sems is not None
        popped = self.nc._tile_sem_poison_stack.pop()
        assert popped is self._sem_poison
        self.nc.clear_and_free_semaphores(list(self.sems.allocated().values()))
        self.nc.all_engine_barrier()


def build_nc():
    nc = bass.Bass("TRN2", target_bir_lowering=False)
    xT = nc.declare_dram_parameter("xT", [E, C], BF16, isOutput=False)
    Wq = nc.declare_dram_parameter("Wq", [E, GF], BF16, isOutput=False)
    Wk = nc.declare_dram_parameter("Wk", [E, GF], BF16, isOutput=False)
    Wv = nc.declare_dram_parameter("Wv", [E, GF], BF16, isOutput=False)
    Wo = nc.declare_dram_parameter("Wo", [GF, E], BF16, isOutput=False)
    tri = nc.declare_dram_parameter("tri", [128, 256], BF16, isOutput=False)
    msk = nc.declare_dram_parameter("mask", [128, 4 * QC], BF16, isOutput=False)
    out = nc.declare_dram_parameter("out", [C, E], F32, isOutput=True)

    xT_t = xT.ap().rearrange("(po pi) f -> pi po f", pi=128)    # [128, 8, C]
    Wq_t = Wq.ap().rearrange("(po pi) f -> pi po f", pi=128)    # [128, 8, GF]
    Wk_t = Wk.ap().rearrange("(po pi) f -> pi po f", pi=128)
    Wv_t = Wv.ap().rearrange("(po pi) f -> pi po f", pi=128)
    Wo_t = Wo.ap().rearrange("(po pi) f -> pi po f", pi=128)    # [128, 4, E]

    with PatchedTC(nc) as tc:
        with contextlib.ExitStack() as ctx:
            consts = ctx.enter_context(tc.tile_pool(name="consts", bufs=1))
            xpool = ctx.enter_context(tc.tile_pool(name="xpool", bufs=1))
            wpool = ctx.enter_context(tc.tile_pool(name="wpool", bufs=1))
            vpool = ctx.enter_context(tc.tile_pool(name="vpool", bufs=1))
            ktpool = ctx.enter_context(tc.tile_pool(name="ktpool", bufs=1))
            qtpool = ctx.enter_context(tc.tile_pool(name="qtpool", bufs=2))
            wtpool = ctx.enter_context(tc.tile_pool(name="wtpool", bufs=2))
            napool = ctx.enter_context(tc.tile_pool(name="napool", bufs=2))
            stgpool = ctx.enter_context(tc.tile_pool(name="stgpool", bufs=1))
            ospool = ctx.enter_context(tc.tile_pool(name="ospool", bufs=3))
            stpool = ctx.enter_context(
                tc.tile_pool(name="stpsum", bufs=2, space="PSUM")
            )
            hidpool = ctx.enter_context(
                tc.tile_pool(name="hidpsum", bufs=1, space="PSUM")
            )
            filpool = ctx.enter_context(
                tc.tile_pool(name="filpsum", bufs=2, space="PSUM")
            )

            tri_sb = consts.tile([128, 256], BF16)
            nc.sync.dma_start(tri_sb[:], tri.ap())
            mask_sb = consts.tile([128, 4 * QC], BF16)
            nc.sync.dma_start(mask_sb[:], msk.ap())

            # weights first (small), then xT token-chunk-major so the first
            # 512 tokens land before the later ones and compute starts early.
            wv_sb = wpool.tile([128, NE, GF], BF16, tag="wv")
            wq_sb = wpool.tile([128, NE, GF], BF16, tag="wq")
            wk_sb = wpool.tile([128, NE, GF], BF16, tag="wk")
            wo_sb = wpool.tile([128, HP, E], BF16, tag="wo")
            nc.sync.dma_start(wv_sb[:], Wv_t[:])
            nc.sync.dma_start(wq_sb[:], Wq_t[:])
            nc.sync.dma_start(wk_sb[:], Wk_t[:])
            nc.sync.dma_start(wo_sb[:], Wo_t[:])

            xT_sb = xpool.tile([128, NE, C], BF16)
            for t in range(NQC):
                for e in range(NE):
                    nc.sync.dma_start(
                        xT_sb[:, e, t * QC : (t + 1) * QC],
                        xT_t[:, e, t * QC : (t + 1) * QC],
                    )

            # v_sb: per head 128 cols = [64 V feats | 64 ones] so the AV
            # matmul's M rows 64:128 accumulate the softmax denominator for
            # free.  Only the ones-halves need the memset; V halves are
            # overwritten by the projection copies.
            v_sb = vpool.tile([128, NKB, 2 * GF], BF16)
            ones_ap = v_sb[:].rearrange("p t (h u) -> p t h u", u=128)[
                :, :, :, 64:128
            ]
            nc.any.memset(ones_ap, 1.0)

            kt_sb = ktpool.tile([128, HP, C], F32R)
            stage_sb = stgpool.tile([128, 2, HP, QC], BF16)

            qts = {}

            # ---- filler tasks: generators yielding once per PE matmul.
            def task_v(tb):
                fil = filpool.tile([128, QC], F32, tag="fil")
                for e in range(NE):
                    nc.tensor.matmul(
                        fil[:],
                        lhsT=xT_sb[:, e, tb * 128 : (tb + 1) * 128],
                        rhs=wv_sb[:, e, :],
                        start=(e == 0),
                        stop=(e == NE - 1),
                    )
                    yield
                dst = v_sb[:, tb, :].rearrange("p (h u) -> p h u", u=128)[
                    :, :, 0:64
                ]
                nc.vector.tensor_copy(
                    dst, fil[:].rearrange("p (h u) -> p h u", u=64)
                )

            def task_qt(qc, hp):
                fil = filpool.tile([128, QC], F32, tag="fil")
                for e in range(NE):
                    nc.tensor.matmul(
                        fil[:],
                        lhsT=wq_sb[:, e, hp * 128 : (hp + 1) * 128],
                        rhs=xT_sb[:, e, qc * QC : (qc + 1) * QC],
                        start=(e == 0),
                        stop=(e == NE - 1),
                    )
                    yield
                qt = qtpool.tile([128, QC], F32R, tag="qt")
                nc.vector.tensor_copy(qt[:], fil[:])
                qts[(qc, hp)] = qt

            def task_kt(qc1, hp):
                fil = filpool.tile([128, QC], F32, tag="fil")
                for e in range(NE):
                    nc.tensor.matmul(
                        fil[:],
                        lhsT=wk_sb[:, e, hp * 128 : (hp + 1) * 128],
                        rhs=xT_sb[:, e, qc1 * QC : (qc1 + 1) * QC],
                        start=(e == 0),
                        stop=(e == NE - 1),
                    )
                    yield
                nc.vector.tensor_copy(
                    kt_sb[:, hp, qc1 * QC : (qc1 + 1) * QC], fil[:]
                )

            def task_o(qcm, j):
                # output projection for row-block 4*qcm+j of q-chunk qcm
                qb = 4 * qcm + j
                for ec in range(2):
                    fil = filpool.tile([128, QC], F32, tag="fil")
                    for f in range(HP):
                        nc.tensor.matmul(
                            fil[:],
                            lhsT=stage_sb[:, qcm % 2, f, j * 128 : (j + 1) * 128],
                            rhs=wo_sb[:, f, ec * QC : (ec + 1) * QC],
                            start=(f == 0),
                            stop=(f == HP - 1),
                        )
                        yield
                    so = ospool.tile([128, QC], F32, tag="so")
                    nc.vector.tensor_copy(so[:], fil[:])
                    nc.sync.dma_start(
                        out.ap()[qb * 128 : (qb + 1) * 128, ec * QC : (ec + 1) * QC],
                        so[:],
                    )

            tasks = deque()

            def pump(n):
                done = 0
                while done < n and tasks:
                    try:
                        next(tasks[0][1])
                        done += 1
                    except StopIteration:
                        tasks.popleft()

            def drain_older(gidx):
                while tasks and tasks[0][0] < gidx:
                    try:
                        next(tasks[0][1])
                    except StopIteration:
                        tasks.popleft()

            # prologue work: V for k-blocks of qc=0, kt for qc=0, qt(0,0)
            for tb in range(4):
                tasks.append((-1, task_v(tb)))
            for hp in range(HP):
                tasks.append((-1, task_kt(0, hp)))
            tasks.append((-1, task_qt(0, 0)))

            # ---- main loop: attention groups with interleaved filler
            gidx = 0
            for qc in range(NQC):
                nkb = 4 * qc + 4
                for hp in range(HP):
                    # queue filler consumed by upcoming groups
                    if hp < HP - 1:
                        tasks.append((gidx, task_qt(qc, hp + 1)))
                    elif qc < NQC - 1:
                        tasks.append((gidx, task_qt(qc + 1, 0)))
                    if qc < NQC - 1:
                        tasks.append((gidx, task_kt(qc + 1, hp)))
                        tasks.append((gidx, task_v(4 * (qc + 1) + hp)))
                    if qc > 0:
                        tasks.append((gidx, task_o(qc - 1, hp)))

                    drain_older(gidx)
                    qt = qts.pop((qc, hp))
                    hid = hidpool.tile([128, 2 * QC], F32, tag="hid")
                    pend = {}
                    for kb in range(nkb + 1):
                        if kb < nkb:
                            dr = kb - 4 * qc
                            # causally-valid q-window of this k-block
                            # (floored at 256 cols: fp32r is 1/4 rate below)
                            qoff = 0 if dr < 1 else min(dr, 2) * 128
                            qoff_s = qoff if TRIM >= 2 else 0
                            qoff_a = qoff if TRIM >= 1 else 0
                            st = stpool.tile([128, 2 * QC], F32, tag="st")
                            nc.tensor.matmul(
                                st[:, qoff_s:QC],
                                lhsT=kt_sb[0:64, hp, kb * KB : (kb + 1) * KB],
                                rhs=qt[0:64, qoff_s:QC],
                                start=True,
                                stop=True,
                            )
                            nc.tensor.matmul(
                                st[:, QC + qoff_s : 2 * QC],
                                lhsT=kt_sb[64:128, hp, kb * KB : (kb + 1) * KB],
                                rhs=qt[64:128, qoff_s:QC],
                                start=True,
                                stop=True,
                            )
                            wt = wtpool.tile([128, 2 * QC], BF16, tag="wt")
                            if TRIM >= 1:
                                st3 = st[:].rearrange("p (a q) -> p a q", a=2)[
                                    :, :, qoff_a:QC
                                ]
                                wt3 = wt[:].rearrange("p (a q) -> p a q", a=2)[
                                    :, :, qoff_a:QC
                                ]
                            else:
                                st3, wt3 = st[:], wt[:]
                            nc.scalar.activation(
                                wt3, st3, mybir.ActivationFunctionType.Exp,
                                scale=0.125,
                            )
                            if dr >= 0:
                                if TRIM >= 1:
                                    # intra-block causal staircase (plus, for
                                    # dr=3, the fully-masked 128 cols kept to
                                    # stay >= 256 wide)
                                    mw = 256 if dr == 3 else 128
                                    moff = 0 if dr == 3 else 128
                                    msrc = tri_sb[:, None, moff : moff + mw]
                                else:
                                    mw = QC
                                    msrc = mask_sb[:, None, dr * QC : (dr + 1) * QC]
                                wtm = wt[:].rearrange("p (a q) -> p a q", a=2)[
                                    :, :, qoff_a : qoff_a + mw
                                ]
                                nc.vector.tensor_tensor(
                                    wtm,
                                    wtm,
                                    msrc.to_broadcast((128, 2, mw)),
                                    mybir.AluOpType.mult,
                                )
                            pend[kb] = (wt, qoff_a)
                        pump(2 if kb < nkb else 4)
                        if kb >= 1:
                            wtp, qoffp = pend.pop(kb - 1)
                            kbp = kb - 1
                            nc.tensor.matmul(
                                hid[:, qoffp:QC],
                                lhsT=v_sb[:, kbp, 2 * hp * 128 : (2 * hp + 1) * 128],
                                rhs=wtp[:, qoffp:QC],
                                start=(kbp == 0),
                                stop=(kbp == nkb - 1),
                                skip_group_check=True,
                            )
                            nc.tensor.matmul(
                                hid[:, QC + qoffp : 2 * QC],
                                lhsT=v_sb[
                                    :, kbp, (2 * hp + 1) * 128 : (2 * hp + 2) * 128
                                ],
                                rhs=wtp[:, QC + qoffp : 2 * QC],
                                start=(kbp == 0),
                                stop=(kbp == nkb - 1),
                                skip_group_check=True,
                            )

                    # 1/rowsum via exp(-ln(rs)) on ACT: ln and exp share one
                    # activation table set (reciprocal doesn't), and DVE's
                    # reciprocal is ~6 cycles/elem.  Rowsums for both heads
                    # sit on rows 64:128 of the merged hid tile.
                    lnr = napool.tile([64, 2 * QC], F32, tag="ln")
                    rec = napool.tile([64, 2 * QC], F32, tag="rec")
                    nc.scalar.activation(
                        lnr[:], hid[64:128, :], mybir.ActivationFunctionType.Ln
                    )
                    nc.scalar.activation(
                        rec[:], lnr[:], mybir.ActivationFunctionType.Exp,
                        scale=-1.0,
                    )
                    nc.vector.tensor_tensor(
                        stage_sb[0:64, qc % 2, hp, :],
                        hid[0:64, 0:QC],
                        rec[:, 0:QC],
                        mybir.AluOpType.mult,
                    )
                    nc.vector.tensor_tensor(
                        stage_sb[64:128, qc % 2, hp, :],
                        hid[0:64, QC : 2 * QC],
                        rec[:, QC : 2 * QC],
                        mybir.AluOpType.mult,
                    )
                    gidx += 1

            # epilogue: output projection of the last q-chunk
            for j in range(HP):
                tasks.append((gidx, task_o(NQC - 1, j)))
            drain_older(gidx + 1)
    return nc


def _make_tri():
    import ml_dtypes

    m = np.zeros((128, 256), dtype=np.float32)
    kk = np.arange(128)[:, None]
    qq = np.arange(128)[None, :]
    m[:, 128:256] = (kk <= qq).astype(np.float32)
    return np.ascontiguousarray(m).astype(ml_dtypes.bfloat16)


def _make_mask():
    import ml_dtypes

    m = np.zeros((128, 4, QC), dtype=np.float32)
    for rr in range(4):
        kk = np.arange(128)[:, None]
        qq = np.arange(QC)[None, :]
        m[:, rr, :] = (128 * rr + kk <= qq).astype(np.float32)
    return np.ascontiguousarray(m.reshape(128, 4 * QC)).astype(ml_dtypes.bfloat16)


def make_in_maps(x, W_q, W_k, W_v, W_o):
    import ml_dtypes

    bf16 = ml_dtypes.bfloat16
    tri = _make_tri()
    mask = _make_mask()
    in_maps = []
    for i in range(N_CORES):
        b, g = i // 2, i % 2
        in_maps.append(
            {
                "xT": np.ascontiguousarray(np.asarray(x)[b].T).astype(bf16),
                "Wq": np.ascontiguousarray(
                    np.asarray(W_q)[:, g * GF : (g + 1) * GF]
                ).astype(bf16),
                "Wk": np.ascontiguousarray(
                    np.asarray(W_k)[:, g * GF : (g + 1) * GF]
                ).astype(bf16),
                "Wv": np.ascontiguousarray(
                    np.asarray(W_v)[:, g * GF : (g + 1) * GF]
                ).astype(bf16),
                "Wo": np.ascontiguousarray(
                    np.asarray(W_o)[g * GF : (g + 1) * GF, :]
                ).astype(bf16),
                "tri": tri,
                "mask": mask,
            }
        )
    return in_maps


def kernel(x, W_q, W_k, W_v, W_o):
    global _CACHED_NC
    from concourse.bass_utils import run_bass_kernel_spmd

    if _CACHED_NC is None:
        _CACHED_NC = build_nc()
    nc = _CACHED_NC

    in_maps = make_in_maps(x, W_q, W_k, W_v, W_o)
    res = run_bass_kernel_spmd(nc, in_maps, core_ids=list(range(N_CORES)))
    out = np.empty((B, C, E), dtype=np.float32)
    for b in range(B):
        out[b] = res.results[2 * b]["out"] + res.results[2 * b + 1]["out"]
    return out


# revision 17
# speedup vs baseline: 1.2030x; 1.0995x over previous
"""Multi-head causal attention (B=4, C=2048, E=1024, H=16, D=64) on 8 trn2 cores.

Sharding: core i = (batch b=i//2, head-group g=i%2).  Each core computes its
batch's attention for 8 heads (512 features) and a partial output projection;
the host sums the two partials per batch (W_o split row-wise).

Single software-pipelined pass, qc-outer / head-pair-inner.  The PE stream for
each attention group (qc, hp) is  [S(kb+1) | filler | AV(kb)]  so the scalar
engine's exp latency is hidden; projection matmuls (V, Q, K for upcoming
groups) and the output projection of the previous q-chunk are pumped into the
stream as filler, so the tensor engine never idles waiting on exp/normalize.
Hidden states stay in SBUF between attention and the output projection (no
DRAM roundtrip).  Diagonal k-blocks only compute the causally-valid q-window
(the S matmul window is floored at 256 columns: fp32r drops to 1/4 rate below
an output free size of 256).
"""

import contextlib
from collections import deque

import numpy as np

import concourse.bass as bass
import concourse.mybir as mybir
import concourse.tile as tile
from concourse.vector_clock import ScopedClock

B, C, E = 4, 2048, 1024
H, D = 16, 64
N_CORES = 8
GF = 512          # features per head-group (8 heads x 64)
HP = 4            # head-pairs per group
QC = 512          # q-chunk width
KB = 128          # k-block width
NQC = C // QC     # 4
NKB = C // KB     # 16
NE = E // 128     # 8 contraction tiles over E
F32 = mybir.dt.float32
F32R = mybir.dt.float32r
BF16 = mybir.dt.bfloat16

# 0: all matmuls full-width (baseline shapes, scheduling change only)
# 1: trim exp/mask/AV to the causally-valid q-window; S full width
# 2: additionally trim the fp32r S matmuls
TRIM = 1

# Q/K projections as fp8e4m3 DoubleRow matmuls (K=256 per call, 2x rate).
# Host pre-packs x and 32*Wq/Wk with E-pairs along the contraction dim;
# the 32x weight scale is compensated in the exp's scale argument.
USE_FP8_QK = False
FP8 = mybir.dt.float8e4

_CACHED_NC = None


class PatchedTC(tile.TileContext):
    """This walrus build caps sync waits per instruction (1 for CTRL, ~2 for
    compute ISA structs).  Hoist excess waits onto same-engine NOPs emitted
    just before the instruction (engine streams execute in order, so the
    semantics are identical), and split the end-of-kernel drain's waits
    across single-wait drain instructions."""

    WAIT_CAP = 1

    def _commit_instruction(self, inst, lazy_reg_writes=True):
        si = getattr(inst, "sync_info", None)
        if (
            si is not None
            and len(si.on_wait) > self.WAIT_CAP
            and getattr(inst, "engine", mybir.EngineType.Unassigned)
            != mybir.EngineType.Unassigned
        ):
            waits = list(si.on_wait)
            keep = waits[: self.WAIT_CAP]
            extra = waits[self.WAIT_CAP :]
            si.on_wait[:] = keep
            for w in extra:
                nop = mybir.InstNoOp(
                    name=f"I-nw{self.nc.next_id()}",
                    engine=inst.engine,
                    bass_nofuse=True,
                    sync_info=mybir.SyncInfo(on_wait=[w], on_update=[]),
                )
                super()._commit_instruction(nop, lazy_reg_writes=False)
        return super()._commit_instruction(inst, lazy_reg_writes)

    def _drain_and_barrier(self, tick_clock, wait_clock):
        carrier = self.nc.sync.drain()
        wait_clock.add_sem_waits(
            carrier.ins, ScopedClock({None: tick_clock.global_clock})
        )
        si = carrier.ins.sync_info
        waits = list(si.on_wait) if si is not None else []
        if len(waits) > 1:
            si.on_wait[:] = waits[:1]
            for w in waits[1:]:
                extra = self.nc.sync.drain()
                extra.ins.sync_info = mybir.SyncInfo(on_wait=[w], on_update=[])
        self.nc.all_engine_barrier()
        assert self.sems is not None
        popped = self.nc._tile_sem_poison_stack.pop()
        assert popped is self._sem_poison
        self.nc.clear_and_free_semaphores(list(self.sems.allocated().values()))
        self.nc.all_engine_barrier()


def build_nc():
    nc = bass.Bass("TRN2", target_bir_lowering=False)
    xT = nc.declare_dram_parameter("xT", [E, C], BF16, isOutput=False)
    Wq = nc.declare_dram_parameter("Wq", [E, GF], BF16, isOutput=False)
    Wk = nc.declare_dram_parameter("Wk", [E, GF], BF16, isOutput=False)
    Wv = nc.declare_dram_parameter("Wv", [E, GF], BF16, isOutput=False)
    Wo = nc.declare_dram_parameter("Wo", [GF, E], BF16, isOutput=False)
    tri = nc.declare_dram_parameter("tri", [128, 256], BF16, isOutput=False)
    msk = nc.declare_dram_parameter("mask", [128, 4 * QC], BF16, isOutput=False)
    if USE_FP8_QK:
        xq8 = nc.declare_dram_parameter("xq8", [128, 4, 2, C], FP8, isOutput=False)
        wq8 = nc.declare_dram_parameter("wq8", [128, 4, 2, GF], FP8, isOutput=False)
        wk8 = nc.declare_dram_parameter("wk8", [128, 4, 2, GF], FP8, isOutput=False)
    out = nc.declare_dram_parameter("out", [C, E], F32, isOutput=True)

    xT_t = xT.ap().rearrange("(po pi) f -> pi po f", pi=128)    # [128, 8, C]
    Wq_t = Wq.ap().rearrange("(po pi) f -> pi po f", pi=128)    # [128, 8, GF]
    Wk_t = Wk.ap().rearrange("(po pi) f -> pi po f", pi=128)
    Wv_t = Wv.ap().rearrange("(po pi) f -> pi po f", pi=128)
    Wo_t = Wo.ap().rearrange("(po pi) f -> pi po f", pi=128)    # [128, 4, E]

    with PatchedTC(nc) as tc:
        with contextlib.ExitStack() as ctx:
            consts = ctx.enter_context(tc.tile_pool(name="consts", bufs=1))
            xpool = ctx.enter_context(tc.tile_pool(name="xpool", bufs=1))
            wpool = ctx.enter_context(tc.tile_pool(name="wpool", bufs=1))
            vpool = ctx.enter_context(tc.tile_pool(name="vpool", bufs=1))
            ktpool = ctx.enter_context(tc.tile_pool(name="ktpool", bufs=1))
            qtpool = ctx.enter_context(tc.tile_pool(name="qtpool", bufs=2))
            wtpool = ctx.enter_context(tc.tile_pool(name="wtpool", bufs=2))
            napool = ctx.enter_context(tc.tile_pool(name="napool", bufs=2))
            stgpool = ctx.enter_context(tc.tile_pool(name="stgpool", bufs=1))
            ospool = ctx.enter_context(tc.tile_pool(name="ospool", bufs=3))
            stpool = ctx.enter_context(
                tc.tile_pool(name="stpsum", bufs=2, space="PSUM")
            )
            hidpool = ctx.enter_context(
                tc.tile_pool(name="hidpsum", bufs=1, space="PSUM")
            )
            filpool = ctx.enter_context(
                tc.tile_pool(name="filpsum", bufs=2, space="PSUM")
            )

            # DMA issue order = first-needed-first: the prologue is gated on
            # wv + the first xT token-chunk (V projection), then wq/wk for
            # the qc=0 kt/qt; wo isn't consumed until the first output
            # projection (~80us in).
            tri_sb = consts.tile([128, 256], BF16)
            nc.sync.dma_start(tri_sb[:], tri.ap())
            mask_sb = consts.tile([128, 4 * QC], BF16)
            if TRIM == 0:
                nc.sync.dma_start(mask_sb[:], msk.ap())

            wv_sb = wpool.tile([128, NE, GF], BF16, tag="wv")
            wq_sb = wpool.tile([128, NE, GF], BF16, tag="wq")
            wk_sb = wpool.tile([128, NE, GF], BF16, tag="wk")
            wo_sb = wpool.tile([128, HP, E], BF16, tag="wo")
            xT_sb = xpool.tile([128, NE, C], BF16)

            if USE_FP8_QK:
                xq8_sb = xpool.tile([128, 4, 2, C], FP8)
                wq8_sb = wpool.tile([128, 4, 2, GF], FP8, tag="wq8")
                wk8_sb = wpool.tile([128, 4, 2, GF], FP8, tag="wk8")

            nc.sync.dma_start(wv_sb[:], Wv_t[:])
            for e in range(NE):
                nc.sync.dma_start(xT_sb[:, e, 0:QC], xT_t[:, e, 0:QC])
            if USE_FP8_QK:
                nc.sync.dma_start(wq8_sb[:], wq8.ap())
                nc.sync.dma_start(wk8_sb[:], wk8.ap())
                nc.sync.dma_start(xq8_sb[:, :, :, 0:QC], xq8.ap()[:, :, :, 0:QC])
            else:
                nc.sync.dma_start(wq_sb[:], Wq_t[:])
                nc.sync.dma_start(wk_sb[:], Wk_t[:])
            for t in range(1, NQC):
                for e in range(NE):
                    nc.sync.dma_start(
                        xT_sb[:, e, t * QC : (t + 1) * QC],
                        xT_t[:, e, t * QC : (t + 1) * QC],
                    )
                if USE_FP8_QK:
                    nc.sync.dma_start(
                        xq8_sb[:, :, :, t * QC : (t + 1) * QC],
                        xq8.ap()[:, :, :, t * QC : (t + 1) * QC],
                    )
            nc.sync.dma_start(wo_sb[:], Wo_t[:])

            # v_sb: per head 128 cols = [64 V feats | 64 ones] so the AV
            # matmul's M rows 64:128 accumulate the softmax denominator for
            # free.  Only the ones-halves need the memset; V halves are
            # overwritten by the projection copies.
            v_sb = vpool.tile([128, NKB, 2 * GF], BF16)
            ones_ap = v_sb[:].rearrange("p t (h u) -> p t h u", u=128)[
                :, :, :, 64:128
            ]
            nc.any.memset(ones_ap, 1.0)

            kt_sb = ktpool.tile([128, HP, C], F32R)
            stage_sb = stgpool.tile([128, 2, HP, QC], BF16)

            qts = {}

            # ---- filler tasks: generators yielding once per PE matmul.
            def task_v(tb):
                fil = filpool.tile([128, QC], F32, tag="fil")
                for e in range(NE):
                    nc.tensor.matmul(
                        fil[:],
                        lhsT=xT_sb[:, e, tb * 128 : (tb + 1) * 128],
                        rhs=wv_sb[:, e, :],
                        start=(e == 0),
                        stop=(e == NE - 1),
                    )
                    yield
                dst = v_sb[:, tb, :].rearrange("p (h u) -> p h u", u=128)[
                    :, :, 0:64
                ]
                nc.vector.tensor_copy(
                    dst, fil[:].rearrange("p (h u) -> p h u", u=64)
                )

            def _proj_chain(fil, w8_sb, w_sb, hp, tok0):
                if USE_FP8_QK:
                    for e4 in range(4):
                        nc.tensor.matmul(
                            fil[:],
                            lhsT=w8_sb[:, e4, :, hp * 128 : (hp + 1) * 128],
                            rhs=xq8_sb[:, e4, :, tok0 : tok0 + QC],
                            start=(e4 == 0),
                            stop=(e4 == 3),
                            perf_mode=mybir.MatmulPerfMode.DoubleRow,
                        )
                        yield
                else:
                    for e in range(NE):
                        nc.tensor.matmul(
                            fil[:],
                            lhsT=w_sb[:, e, hp * 128 : (hp + 1) * 128],
                            rhs=xT_sb[:, e, tok0 : tok0 + QC],
                            start=(e == 0),
                            stop=(e == NE - 1),
                        )
                        yield

            def task_qt(qc, hp):
                fil = filpool.tile([128, QC], F32, tag="fil")
                yield from _proj_chain(
                    fil, wq8_sb if USE_FP8_QK else None, wq_sb, hp, qc * QC
                )
                qt = qtpool.tile([128, QC], F32R, tag="qt")
                nc.vector.tensor_copy(qt[:], fil[:])
                qts[(qc, hp)] = qt

            def task_kt(qc1, hp):
                fil = filpool.tile([128, QC], F32, tag="fil")
                yield from _proj_chain(
                    fil, wk8_sb if USE_FP8_QK else None, wk_sb, hp, qc1 * QC
                )
                nc.vector.tensor_copy(
                    kt_sb[:, hp, qc1 * QC : (qc1 + 1) * QC], fil[:]
                )

            def task_o(qcm, j):
                # output projection for row-block 4*qcm+j of q-chunk qcm
                qb = 4 * qcm + j
                for ec in range(2):
                    fil = filpool.tile([128, QC], F32, tag="fil")
                    for f in range(HP):
                        nc.tensor.matmul(
                            fil[:],
                            lhsT=stage_sb[:, qcm % 2, f, j * 128 : (j + 1) * 128],
                            rhs=wo_sb[:, f, ec * QC : (ec + 1) * QC],
                            start=(f == 0),
                            stop=(f == HP - 1),
                        )
                        yield
                    so = ospool.tile([128, QC], F32, tag="so")
                    nc.vector.tensor_copy(so[:], fil[:])
                    nc.sync.dma_start(
                        out.ap()[qb * 128 : (qb + 1) * 128, ec * QC : (ec + 1) * QC],
                        so[:],
                    )

            tasks = deque()

            def pump(n):
                done = 0
                while done < n and tasks:
                    try:
                        next(tasks[0][1])
                        done += 1
                    except StopIteration:
                        tasks.popleft()

            def drain_older(gidx):
                while tasks and tasks[0][0] < gidx:
                    try:
                        next(tasks[0][1])
                    except StopIteration:
                        tasks.popleft()

            # prologue work: V for k-blocks of qc=0, kt for qc=0, qt(0,0)
            for tb in range(4):
                tasks.append((-1, task_v(tb)))
            for hp in range(HP):
                tasks.append((-1, task_kt(0, hp)))
            tasks.append((-1, task_qt(0, 0)))

            # ---- main loop: attention groups with interleaved filler
            gidx = 0
            for qc in range(NQC):
                nkb = 4 * qc + 4
                for hp in range(HP):
                    # queue filler consumed by upcoming groups
                    if hp < HP - 1:
                        tasks.append((gidx, task_qt(qc, hp + 1)))
                    elif qc < NQC - 1:
                        tasks.append((gidx, task_qt(qc + 1, 0)))
                    if qc < NQC - 1:
                        tasks.append((gidx, task_kt(qc + 1, hp)))
                        tasks.append((gidx, task_v(4 * (qc + 1) + hp)))
                    if qc > 0:
                        tasks.append((gidx, task_o(qc - 1, hp)))

                    drain_older(gidx)
                    qt = qts.pop((qc, hp))
                    hid = hidpool.tile([128, 2 * QC], F32, tag="hid")
                    pend = {}
                    for kb in range(nkb + 1):
                        if kb < nkb:
                            dr = kb - 4 * qc
                            # causally-valid q-window of this k-block
                            # (floored at 256 cols: fp32r is 1/4 rate below)
                            qoff = 0 if dr < 1 else min(dr, 2) * 128
                            qoff_s = qoff if TRIM >= 2 else 0
                            qoff_a = qoff if TRIM >= 1 else 0
                            st = stpool.tile([128, 2 * QC], F32, tag="st")
                            nc.tensor.matmul(
                                st[:, qoff_s:QC],
                                lhsT=kt_sb[0:64, hp, kb * KB : (kb + 1) * KB],
                                rhs=qt[0:64, qoff_s:QC],
                                start=True,
                                stop=True,
                            )
                            nc.tensor.matmul(
                                st[:, QC + qoff_s : 2 * QC],
                                lhsT=kt_sb[64:128, hp, kb * KB : (kb + 1) * KB],
                                rhs=qt[64:128, qoff_s:QC],
                                start=True,
                                stop=True,
                            )
                            wt = wtpool.tile([128, 2 * QC], BF16, tag="wt")
                            if TRIM >= 1:
                                st3 = st[:].rearrange("p (a q) -> p a q", a=2)[
                                    :, :, qoff_a:QC
                                ]
                                wt3 = wt[:].rearrange("p (a q) -> p a q", a=2)[
                                    :, :, qoff_a:QC
                                ]
                            else:
                                st3, wt3 = st[:], wt[:]
                            nc.scalar.activation(
                                wt3, st3, mybir.ActivationFunctionType.Exp,
                                scale=0.125,
                            )
                            if dr >= 0:
                                if TRIM >= 1:
                                    # intra-block causal staircase (plus, for
                                    # dr=3, the fully-masked 128 cols kept to
                                    # stay >= 256 wide)
                                    mw = 256 if dr == 3 else 128
                                    moff = 0 if dr == 3 else 128
                                    msrc = tri_sb[:, None, moff : moff + mw]
                                else:
                                    mw = QC
                                    msrc = mask_sb[:, None, dr * QC : (dr + 1) * QC]
                                wtm = wt[:].rearrange("p (a q) -> p a q", a=2)[
                                    :, :, qoff_a : qoff_a + mw
                                ]
                                nc.vector.tensor_tensor(
                                    wtm,
                                    wtm,
                                    msrc.to_broadcast((128, 2, mw)),
                                    mybir.AluOpType.mult,
                                )
                            pend[kb] = (wt, qoff_a)
                        # pump rate tapers with qc so late groups keep a
                        # filler cushion for the group-boundary WAR gap
                        if kb >= nkb:
                            pump(4)
                        elif qc == 3:
                            pump(1 if kb % 2 == 1 else 0)
                        else:
                            pump((4, 3, 2)[qc])
                        if kb >= 1:
                            wtp, qoffp = pend.pop(kb - 1)
                            kbp = kb - 1
                            nc.tensor.matmul(
                                hid[:, qoffp:QC],
                                lhsT=v_sb[:, kbp, 2 * hp * 128 : (2 * hp + 1) * 128],
                                rhs=wtp[:, qoffp:QC],
                                start=(kbp == 0),
                                stop=(kbp == nkb - 1),
                                skip_group_check=True,
                            )
                            nc.tensor.matmul(
                                hid[:, QC + qoffp : 2 * QC],
                                lhsT=v_sb[
                                    :, kbp, (2 * hp + 1) * 128 : (2 * hp + 2) * 128
                                ],
                                rhs=wtp[:, QC + qoffp : 2 * QC],
                                start=(kbp == 0),
                                stop=(kbp == nkb - 1),
                                skip_group_check=True,
                            )

                    # 1/rowsum via exp(-ln(rs)) on ACT: ln and exp share one
                    # activation table set (reciprocal doesn't), and DVE's
                    # reciprocal is ~6 cycles/elem.  Rowsums for both heads
                    # sit on rows 64:128 of the merged hid tile.
                    lnr = napool.tile([64, 2 * QC], F32, tag="ln")
                    rec = napool.tile([64, 2 * QC], F32, tag="rec")
                    nc.scalar.activation(
                        lnr[:], hid[64:128, :], mybir.ActivationFunctionType.Ln
                    )
                    nc.scalar.activation(
                        rec[:], lnr[:], mybir.ActivationFunctionType.Exp,
                        scale=-1.0,
                    )
                    nc.vector.tensor_tensor(
                        stage_sb[0:64, qc % 2, hp, :],
                        hid[0:64, 0:QC],
                        rec[:, 0:QC],
                        mybir.AluOpType.mult,
                    )
                    nc.vector.tensor_tensor(
                        stage_sb[64:128, qc % 2, hp, :],
                        hid[0:64, QC : 2 * QC],
                        rec[:, QC : 2 * QC],
                        mybir.AluOpType.mult,
                    )
                    gidx += 1

            # epilogue: output projection of the last q-chunk
            for j in range(HP):
                tasks.append((gidx, task_o(NQC - 1, j)))
            drain_older(gidx + 1)
    return nc


def _make_tri():
    import ml_dtypes

    m = np.zeros((128, 256), dtype=np.float32)
    kk = np.arange(128)[:, None]
    qq = np.arange(128)[None, :]
    m[:, 128:256] = (kk <= qq).astype(np.float32)
    return np.ascontiguousarray(m).astype(ml_dtypes.bfloat16)


def _make_mask():
    import ml_dtypes

    m = np.zeros((128, 4, QC), dtype=np.float32)
    for rr in range(4):
        kk = np.arange(128)[:, None]
        qq = np.arange(QC)[None, :]
        m[:, rr, :] = (128 * rr + kk <= qq).astype(np.float32)
    return np.ascontiguousarray(m.reshape(128, 4 * QC)).astype(ml_dtypes.bfloat16)


def make_in_maps(x, W_q, W_k, W_v, W_o):
    import ml_dtypes

    bf16 = ml_dtypes.bfloat16
    tri = _make_tri()
    mask = _make_mask()
    in_maps = []
    for i in range(N_CORES):
        b, g = i // 2, i % 2
        in_maps.append(
            {
                "xT": np.ascontiguousarray(np.asarray(x)[b].T).astype(bf16),
                "Wq": np.ascontiguousarray(
                    np.asarray(W_q)[:, g * GF : (g + 1) * GF]
                ).astype(bf16),
                "Wk": np.ascontiguousarray(
                    np.asarray(W_k)[:, g * GF : (g + 1) * GF]
                ).astype(bf16),
                "Wv": np.ascontiguousarray(
                    np.asarray(W_v)[:, g * GF : (g + 1) * GF]
                ).astype(bf16),
                "Wo": np.ascontiguousarray(
                    np.asarray(W_o)[g * GF : (g + 1) * GF, :]
                ).astype(bf16),
                "tri": tri,
                "mask": mask,
            }
        )
    return in_maps


def kernel(x, W_q, W_k, W_v, W_o):
    global _CACHED_NC
    from concourse.bass_utils import run_bass_kernel_spmd

    if _CACHED_NC is None:
        _CACHED_NC = build_nc()
    nc = _CACHED_NC

    in_maps = make_in_maps(x, W_q, W_k, W_v, W_o)
    res = run_bass_kernel_spmd(nc, in_maps, core_ids=list(range(N_CORES)))
    out = np.empty((B, C, E), dtype=np.float32)
    for b in range(B):
        out[b] = res.results[2 * b]["out"] + res.results[2 * b + 1]["out"]
    return out
